# revision 1
# baseline (speedup 1.0000x reference)
"""Trainium2 Bass kernel for the masked-attention-with-relative-bias module.

Contract: kernel(**inputs) takes FULL unsharded numpy inputs and returns the
FULL [16, 1024, 512] float32 output. Internally shards the batch dim over 8
NeuronCores (2 batches/core, embarrassingly parallel, no collectives).

Algorithm notes (per core, B_loc=2, L=1024, C=512, H=8, d=64):
  - Everything contracts over features, so x is transposed once on-chip (PE
    transpose against an identity) into xT fp16; q/k are produced
    feature-major (qT/kT), v token-major, all via fp16 matmuls against xT
    with fp32 PSUM accumulation.
  - Scores are computed transposed: sT[key, query] = kT.T @ qT. The softmax
    key-mask (0 / -30000) rides the Exp activation's per-partition bias
    operand (keys live on partitions), costing nothing. The two batches'
    tile streams are interleaved per key-tile to keep all engines fed.
  - rel_bias is a pointwise MLP of a Toeplitz matrix, so only the 2045
    distinct values run through the MLP on device (leaky_relu built as
    0.6x + 0.4|x| because the ACT table's Lrelu ignores the alpha arg);
    exp(rel) is expanded into [key, query] fp16 tiles by a strided DMA
    from a DRAM table written with row stride 4097 and read with row
    stride 4096 (the stride mismatch yields the per-partition diagonal
    shift with all-positive AP steps) and folded in post-exp:
    exp(s + rel) = exp(s) * exp(rel).
  - The softmax denominator falls out of the PV matmul via an appended
    ones-column on v (out has 65 rows, row 64 = sum of probabilities).
  - Normalization: 1/den as exp(-ln(den)) on ACT, partition-broadcast via
    a DRAM bounce (write [1,L], re-read with partition-step-0 AP), one DVE
    multiply that also casts to fp16 in the aoT layout the output
    projection wants as its stationary operand.
"""

import os

import numpy as np

import concourse.bass as bass
import concourse.mybir as mybir
import concourse.tile as tile
from concourse import bass_utils
from concourse.masks import make_identity

F32 = mybir.dt.float32
F32R = mybir.dt.float32r
FP16 = mybir.dt.float16
BF16 = mybir.dt.bfloat16

B, L, C, H, D = 16, 1024, 512, 8, 64
NCORES = 8
B_LOC = B // NCORES          # batches per core
T = B_LOC * L                # tokens per core
NPT = L // 128               # key partition-tiles per batch
SLOPE = 8.0
NEG_SLOPE = 0.2
NREL = 2 * (L - 2) + 1       # 2045 distinct rel values
MASK_NEG = -30000.0
EXPROW = 524288              # per-head flat size of the shifted table

_compiled = {"nc": None}


def _build_kernel():
    nc = bass.Bass("TRN2", target_bir_lowering=False, debug=False,
                   enable_asserts=False)

    x_d = nc.dram_tensor("x", [T, C], F32, kind="ExternalInput")
    wqkv_d = nc.dram_tensor("wqkv", [C, 3 * C], F32, kind="ExternalInput")
    w1_d = nc.dram_tensor("w1", [1, D], F32, kind="ExternalInput")
    b1_d = nc.dram_tensor("b1", [D, 1], F32, kind="ExternalInput")
    w2_d = nc.dram_tensor("w2", [D, H], F32, kind="ExternalInput")
    wout_d = nc.dram_tensor("wout", [C, C], F32, kind="ExternalInput")
    trev_d = nc.dram_tensor("trev", [1, 2048], F32, kind="ExternalInput")
    mbias_d = nc.dram_tensor("mbias", [128, B_LOC * NPT], F32,
                             kind="ExternalInput")
    out_d = nc.dram_tensor("out", [T, C], F32, kind="ExternalOutput")
    # rel-bias values per head, reversed layout + guard padding (f32r)
    ervr_d = nc.dram_tensor("ervr", [H, 4096], FP16)
    # per-head diagonal-shifted expansion table: written with row stride
    # 4097, read with row stride 4096, so partition p sees a -p shift with
    # all-positive AP steps
    ervx_d = nc.dram_tensor("ervx", [H, EXPROW], FP16)
    # softmax reciprocal rows bounced through DRAM for partition-broadcast
    recip_d = nc.dram_tensor("recip", [B_LOC * H, L], F32)

    with tile.TileContext(nc) as tc:
        _body(nc, tc, x_d, wqkv_d, w1_d, b1_d, w2_d, wout_d, trev_d,
              mbias_d, out_d, ervr_d, ervx_d, recip_d)
    _split_range_clear(nc)
    _split_dma_waits(nc)
    return nc


def _split_range_clear(nc):
    """walrus in this toolchain rejects EVENT_SEMAPHORE_RANGE_CLEAR over a
    wide semaphore range ("ISA wrong length"). Split into <=8-wide
    subrange clears."""
    import concourse.bass_isa as bass_isa

    for fn in nc.m.functions:
        for blk in fn.blocks:
            out = []
            for inst in blk.instructions:
                if (isinstance(inst, mybir.InstISA)
                        and inst.op_name == "EVENT_SEMAPHORE_RANGE_CLEAR"
                        and inst.ant_dict["range_last"]
                        - inst.ant_dict["range_first"] >= 8):
                    first = inst.ant_dict["range_first"]
                    last = inst.ant_dict["range_last"]
                    si = inst.sync_info
                    k = 0
                    for lo in range(first, last + 1, 8):
                        hi = min(lo + 7, last)
                        ant = {"mode": inst.ant_dict["mode"],
                               "range_first": lo, "range_last": hi}
                        instr, fixups = bass_isa.isa_struct(
                            nc.isa, inst.isa_opcode, ant)
                        ni = mybir.InstISA(
                            name=f"{inst.name}-rc{k}",
                            isa_opcode=inst.isa_opcode,
                            engine=inst.engine,
                            instr=instr,
                            op_name=inst.op_name,
                            ins=[], outs=[],
                            ant_dict=ant,
                            verify=False,
                            ant_isa_is_sequencer_only=True,
                        )
                        if si is not None and k == 0:
                            ni.sync_info = mybir.SyncInfo(
                                on_wait=list(si.on_wait), on_update=[])
                        if si is not None and lo + 8 > last:
                            prev = ni.sync_info
                            ni.sync_info = mybir.SyncInfo(
                                on_wait=list(prev.on_wait) if prev else [],
                                on_update=list(si.on_update))
                        out.append(ni)
                        k += 1
                else:
                    out.append(inst)
            blk.instructions = out


def _split_dma_waits(nc):
    """walrus in this toolchain rejects instructions carrying more than one
    sync wait ("Too many sync wait commands"). Hoist all but one wait onto
    standalone EventSemaphore instructions (<=2 waits each) placed
    immediately before the instruction in the same (in-order) engine
    stream — semantics are unchanged."""
    for fn in nc.m.functions:
        for blk in fn.blocks:
            out = []
            for inst in blk.instructions:
                si = inst.sync_info
                if (si is not None and len(si.on_wait) > 1
                        and not isinstance(inst, mybir.InstEventSemaphore)):
                    hoist = list(si.on_wait[:-1])
                    for j in range(0, len(hoist), 2):
                        ev = mybir.InstEventSemaphore(
                            name=f"{inst.name}-hw{j}", ins=[], outs=[])
                        ev.engine = inst.engine
                        ev.sync_info = mybir.SyncInfo(
                            on_wait=hoist[j:j + 2], on_update=[])
                        out.append(ev)
                    inst.sync_info = mybir.SyncInfo(
                        on_wait=[si.on_wait[-1]],
                        on_update=list(si.on_update))
                out.append(inst)
            blk.instructions = out


def _body(nc, tc, x_d, wqkv_d, w1_d, b1_d, w2_d, wout_d, trev_d, mbias_d,
          out_d, ervr_d, ervx_d, recip_d):
    AF = mybir.ActivationFunctionType
    NT = T // 128  # token tiles (16)
    KC = C // 128  # contraction chunks over C (4)

    with tc.tile_pool(name="persist", bufs=1) as persist:
        # long-lived SBUF tensors (all 16-bit compute tensors are fp16)
        qT = [persist.tile([128, T], FP16, name=f"qT{i}", tag=f"qT{i}")
              for i in range(KC)]
        kT = [persist.tile([128, T], FP16, name=f"kT{i}", tag=f"kT{i}")
              for i in range(KC)]
        aoT = [persist.tile([128, T], FP16, name=f"aoT{i}", tag=f"aoT{i}")
               for i in range(KC)]
        wo = [persist.tile([128, C], FP16, name=f"wo{i}", tag=f"wo{i}")
              for i in range(KC)]
        # v with ones column: [128 keys, (b,kc) x head x 65]
        vhat = persist.tile([128, NT * H * 65], FP16, name="vhat", tag="vhat")
        vhat_r = vhat.rearrange("p (t h c) -> p t h c", t=NT, h=H)
        mbS = persist.tile([128, B_LOC * NPT], F32, name="mbS", tag="mbS")
        fixS = persist.tile([H, L], FP16, name="fixS", tag="fixS")

        nc.sync.dma_start(out=mbS, in_=mbias_d[:, :])
        nc.vector.memset(vhat, 1.0)

        # ---- rel-bias MLP on the 2045 distinct Toeplitz values ----
        with (
            tc.tile_pool(name="mlp", bufs=1) as mlp,
            tc.tile_pool(name="mlpp", bufs=1, space="PSUM") as mlpp,
        ):
            trS = mlp.tile([1, 2048], F32, name="trS", tag="trS")
            w1S = mlp.tile([1, D], F32, name="w1S", tag="w1S")
            b1S = mlp.tile([D, 1], F32, name="b1S", tag="b1S")
            w2S = mlp.tile([D, H], F32, name="w2S", tag="w2S")
            nc.sync.dma_start(out=trS, in_=trev_d[:, :])
            nc.sync.dma_start(out=w1S, in_=w1_d[:, :])
            nc.sync.dma_start(out=b1S, in_=b1_d[:, :])
            nc.sync.dma_start(out=w2S, in_=w2_d[:, :])

            h1P = mlpp.tile([D, 2048], F32, name="h1P", tag="h1P")
            for nb in range(4):
                nc.tensor.matmul(h1P[:, bass.ts(nb, 512)], w1S,
                                 trS[:, bass.ts(nb, 512)],
                                 start=True, stop=True)
            # leaky_relu(pre, 0.2) = 0.6*pre + 0.4*|pre| (the ACT table's
            # Lrelu entry has a baked-in alpha of 0.01 and ignores the
            # alpha argument)
            b1s6 = mlp.tile([D, 1], F32, name="b1s6", tag="b1s6")
            nc.vector.tensor_scalar_mul(b1s6, b1S, (1.0 + NEG_SLOPE) / 2)
            a1 = mlp.tile([D, 2048], F32, name="a1", tag="a1")
            nc.scalar.activation(a1, h1P, AF.Abs, bias=b1S, scale=1.0)
            p1 = mlp.tile([D, 2048], F32, name="p1", tag="p1")
            nc.scalar.activation(p1, h1P, AF.Identity, bias=b1s6,
                                 scale=(1.0 + NEG_SLOPE) / 2)
            hS = mlp.tile([D, 2048], F32, name="hS", tag="hS")
            nc.vector.scalar_tensor_tensor(
                out=hS, in0=a1, scalar=(1.0 - NEG_SLOPE) / 2, in1=p1,
                op0=mybir.AluOpType.mult, op1=mybir.AluOpType.add)
            rvP = mlpp.tile([H, 2048], F32, name="rvP", tag="rvP")
            for nb in range(4):
                nc.tensor.matmul(rvP[:, bass.ts(nb, 512)], w2S,
                                 hS[:, bass.ts(nb, 512)],
                                 start=True, stop=True)
            ervS = mlp.tile([H, 2048], FP16, name="ervS", tag="ervS")
            nc.scalar.activation(ervS, rvP, AF.Exp)
            erv0 = mlp.tile([H, 1], F32, name="erv0", tag="erv0")
            nc.scalar.activation(erv0, rvP[:, 1022:1023], AF.Exp)
            # fixS[h, :] = exp(rv0[h]) everywhere (bias row for key j=0)
            nc.scalar.activation(fixS, ervS[:, 0:L], AF.Identity, bias=erv0,
                                 scale=0.0)
            nc.gpsimd.dma_start(out=ervr_d[:, 2:2048], in_=ervS[:, 0:2046])
            nc.gpsimd.dma_start(out=ervr_d[:, 1:2], in_=ervS[:, 0:1])
            # build the diagonal-shifted table: broadcast each head's row
            # 128x into DRAM at row stride 4097
            ervr_t = ervr_d.ap().tensor
            ervx_t = ervx_d.ap().tensor
            for h in range(H):
                for pz in range(2):
                    src = bass.AP(tensor=ervr_t, offset=h * 4096 + 2,
                                  ap=[[0, 64], [1, 2046]])
                    dst = bass.AP(
                        tensor=ervx_t,
                        offset=h * EXPROW + pz * 64 * 4097 + 2,
                        ap=[[4097, 64], [1, 2046]])
                    nc.gpsimd.dma_start(out=dst, in_=src)

        with tc.tile_pool(name="wstage", bufs=2) as wstage:
            for i in range(KC):
                wf2 = wstage.tile([128, C], F32, name="wf2", tag="wf2")
                nc.scalar.dma_start(out=wf2, in_=wout_d[bass.ts(i, 128), :])
                nc.vector.tensor_copy(wo[i], wf2)

        with tc.tile_pool(name="xtpool", bufs=1) as xtpool:
            xT = [xtpool.tile([128, T], FP16, name=f"xT{i}", tag=f"xT{i}")
                  for i in range(KC)]
            wq = [xtpool.tile([128, 3 * C], FP16, name=f"wq{i}",
                              tag=f"wq{i}") for i in range(KC)]

            identF = xtpool.tile([128, 128], F32, name="identF",
                                 tag="identF")
            make_identity(nc, identF)
            identH = xtpool.tile([128, 128], FP16, name="identH",
                                 tag="identH")
            nc.vector.tensor_copy(identH, identF)

            # ---- load + cast W_qkv, transpose x into xT (fp16, PE) ----
            with (
                tc.tile_pool(name="xstage", bufs=4) as xstage,
                tc.tile_pool(name="xtp", bufs=4, space="PSUM") as xtp,
            ):
                for i in range(KC):
                    wf = xstage.tile([128, 3 * C], F32, name="wf", tag="wf")
                    nc.scalar.dma_start(out=wf, in_=wqkv_d[bass.ts(i, 128), :])
                    nc.vector.tensor_copy(wq[i], wf)
                for tt in range(NT):
                    xS = xstage.tile([128, C], F32, name="xS", tag="xS")
                    nc.scalar.dma_start(out=xS, in_=x_d[bass.ts(tt, 128), :])
                    xh = xstage.tile([128, C], FP16, name="xh", tag="xh")
                    nc.vector.tensor_copy(xh, xS)
                    for kc in range(KC):
                        tp = xtp.tile([128, 128], FP16, name="tp", tag="tp")
                        nc.tensor.transpose(tp, xh[:, bass.ts(kc, 128)],
                                            identH)
                        nc.vector.tensor_copy(xT[kc][:, bass.ts(tt, 128)], tp)

            # ---- qT / kT projections (feature-major, fp16) ----
            with tc.tile_pool(name="projp", bufs=2, space="PSUM") as projp:
                for ft in range(8):  # feature tiles over 1024 q|k features
                    qkP = projp.tile([128, T], F32, name="qkP", tag="qkP")
                    for nb in range(4):
                        for kc in range(KC):
                            nc.tensor.matmul(
                                qkP[:, bass.ts(nb, 512)],
                                wq[kc][:, bass.ts(ft, 128)],
                                xT[kc][:, bass.ts(nb, 512)],
                                start=(kc == 0), stop=(kc == KC - 1))
                    if ft < 4:
                        nc.vector.tensor_copy(qT[ft], qkP)
                    else:
                        nc.vector.tensor_copy(kT[ft - 4], qkP)

                # ---- v projection (token-major) into vhat ----
                for tt in range(NT):
                    vP = projp.tile([128, C], F32, name="vP", tag="qkP")
                    for kc in range(KC):
                        nc.tensor.matmul(vP, xT[kc][:, bass.ts(tt, 128)],
                                         wq[kc][:, 1024:1536],
                                         start=(kc == 0), stop=(kc == KC - 1))
                    vP_r = vP.rearrange("p (h c) -> p h c", h=H)
                    nc.vector.tensor_copy(vhat_r[:, tt, :, 0:D], vP_r)

        # ---- attention ----
        with (
            tc.tile_pool(name="epool", bufs=24) as epool,
            tc.tile_pool(name="p2pool", bufs=16) as p2pool,
            tc.tile_pool(name="rpool", bufs=2) as rpool,
            tc.tile_pool(name="rbpool", bufs=2) as rbpool,
            tc.tile_pool(name="scp", bufs=2, space="PSUM") as scp,
            tc.tile_pool(name="aop", bufs=2, space="PSUM") as aop,
        ):
            ervx_t = ervx_d.ap().tensor
            recip_t = recip_d.ap().tensor
            for h in range(H):
                tq, po = h // 2, 64 * (h % 2)
                etiles = []
                for pt in range(NPT):
                    E = epool.tile([128, L], FP16, name="E", tag="E")
                    src = bass.AP(tensor=ervx_t,
                                  offset=h * EXPROW + 1024 - pt * 128,
                                  ap=[[4096, 128], [1, L]])
                    nc.sync.dma_start(out=E, in_=src)
                    if pt == 0:
                        # key j=0 has constant bias exp(f(0))
                        nc.sync.dma_start(out=E[0:1, :], in_=fixS[h:h + 1, :])
                    # query i=0 bias is constant across keys: any constant
                    # works (softmax-invariant); use 1
                    nc.vector.memset(E[:, 0:1], 1.0)
                    etiles.append(E)
                aos = []
                for b in range(B_LOC):
                    ao = aop.tile([65, L], F32, name=f"ao{b}", tag="ao")
                    aos.append(ao)
                for pt in range(NPT):
                    for b in range(B_LOC):
                        sc = scp.tile([128, L], F32, name="sc", tag="sc")
                        lhsT = kT[tq][po:po + D,
                                      b * L + pt * 128:b * L + (pt + 1) * 128]
                        for nb in range(2):
                            nc.tensor.matmul(
                                sc[:, bass.ts(nb, 512)], lhsT,
                                qT[tq][po:po + D,
                                       b * L + nb * 512:
                                       b * L + (nb + 1) * 512],
                                start=True, stop=True)
                        pp = p2pool.tile([128, L], FP16, name="pp", tag="pp")
                        nc.scalar.activation(
                            pp, sc, AF.Exp,
                            bias=mbS[:, b * NPT + pt:b * NPT + pt + 1],
                            scale=1.0)
                        nc.vector.tensor_mul(pp, pp, etiles[pt])
                        vv = vhat_r[:, b * NPT + pt, h, 0:65]
                        for nb in range(2):
                            nc.tensor.matmul(
                                aos[b][:, bass.ts(nb, 512)], vv,
                                pp[:, bass.ts(nb, 512)],
                                start=(pt == 0), stop=(pt == NPT - 1))
                for b in range(B_LOC):
                    ao = aos[b]
                    lden = rpool.tile([1, L], F32, name="lden", tag="lden")
                    nc.scalar.activation(lden, ao[64:65, :], AF.Ln)
                    recip = rpool.tile([1, L], F32, name="recip", tag="recip")
                    nc.scalar.activation(recip, lden, AF.Exp, scale=-1.0)
                    row = b * H + h
                    nc.sync.dma_start(out=recip_d[row:row + 1, :], in_=recip)
                    rbc = rbpool.tile([D, L], F32, name="rbc", tag="rbc")
                    rsrc = bass.AP(tensor=recip_t, offset=row * L,
                                   ap=[[0, D], [1, L]])
                    nc.gpsimd.dma_start(out=rbc, in_=rsrc)
                    nc.vector.tensor_mul(
                        aoT[tq][po:po + D, b * L:(b + 1) * L],
                        ao[0:D, :], rbc)

        # ---- output projection (fp16) ----
        with (
            tc.tile_pool(name="fpool", bufs=4, space="PSUM") as fpool,
            tc.tile_pool(name="opool", bufs=4) as opool,
        ):
            for tt in range(NT):
                fP = fpool.tile([128, C], F32, name="fP", tag="fP")
                for kc in range(KC):
                    nc.tensor.matmul(fP, aoT[kc][:, bass.ts(tt, 128)], wo[kc],
                                     start=(kc == 0), stop=(kc == KC - 1))
                oS = opool.tile([128, C], F32, name="oS", tag="oS")
                nc.vector.tensor_copy(oS, fP)
                nc.scalar.dma_start(out=out_d[bass.ts(tt, 128), :], in_=oS)


def _host_inputs(x, attn_mask, W_qkv, W1, b1, W2, W_out):
    """Build per-core input maps (pure reshapes / constant folding)."""
    x = np.ascontiguousarray(x, dtype=np.float32)
    W_qkv = np.ascontiguousarray(W_qkv, dtype=np.float32)

    wqkv_scaled = W_qkv.copy()
    wqkv_scaled[:, :C] *= D ** -0.5

    # distinct Toeplitz values of the rel table, reversed: trev[k]=g(1022-k)
    n = L - 1
    step = SLOPE / (n - 1)
    delta = (1022 - np.arange(NREL)).astype(np.float64)
    rel = delta * step
    g = np.sign(rel) * np.log2(np.abs(rel) + 1.0) / np.log2(SLOPE + 1.0)
    trev = np.zeros((1, 2048), dtype=np.float32)
    trev[0, :NREL] = g.astype(np.float32)

    # mask bias: [B, L] with col 0 always valid, laid out [128, b*NPT+pt]
    m = np.concatenate([np.ones((B, 1), dtype=bool),
                        np.asarray(attn_mask, dtype=bool)], axis=1)
    mb = np.where(m, 0.0, MASK_NEG).astype(np.float32)

    common = {
        "wqkv": wqkv_scaled,
        "w1": np.ascontiguousarray(W1, dtype=np.float32),
        "b1": np.ascontiguousarray(b1, dtype=np.float32).reshape(D, 1),
        "w2": np.ascontiguousarray(W2, dtype=np.float32),
        "wout": np.ascontiguousarray(W_out, dtype=np.float32),
        "trev": trev,
    }
    in_maps = []
    for core in range(NCORES):
        b0 = core * B_LOC
        mbias = np.empty((128, B_LOC * NPT), dtype=np.float32)
        for bl in range(B_LOC):
            mbias[:, bl * NPT:(bl + 1) * NPT] = (
                mb[b0 + bl].reshape(NPT, 128).T)
        in_maps.append({
            **common,
            "x": np.ascontiguousarray(
                x[b0:b0 + B_LOC].reshape(T, C)),
            "mbias": mbias,
        })
    return in_maps


last_exec_time_ns = None


def kernel(x, attn_mask, W_qkv, W1, b1, W2, W_out):
    global last_exec_time_ns
    if _compiled["nc"] is None:
        _compiled["nc"] = _build_kernel()
    nc = _compiled["nc"]

    in_maps = _host_inputs(x, attn_mask, W_qkv, W1, b1, W2, W_out)
    trace = os.environ.get("KERNEL_TRACE", "0") == "1"
    res = bass_utils.run_bass_kernel_spmd(
        nc, in_maps, core_ids=list(range(NCORES)), trace=trace)
    last_exec_time_ns = res.exec_time_ns

    out = np.concatenate(
        [r["out"].reshape(B_LOC, L, C) for r in res.results], axis=0)
    return out

